# revision 28
# baseline (speedup 1.0000x reference)
"""Multi-head attention kernel for Trainium2, 8 NeuronCores, data-parallel over batch.

Problem (matches the reference nn.Module):
  B=8, S=1024, D_IN=D_OUT=1024, H=16, D_K=64, fp32 in/out.
  q/k/v = Linear(x) per input; scores = q k^T / sqrt(64); attn = softmax;
  out = (attn v) heads-concatenated -> [B, S*D_OUT].

Strategy:
  - One batch element per core (8 cores). No collectives.
  - All matmul operands bf16 (PSUM f32). Host pre-transposes inputs.
  - On-chip layouts:
      kT/qT [P, KT, S]  (o = p + 128*t; head h lives at t=h//2,
                         partitions (h%2)*64..+64)
      vP    [P, KT, H*(DK+1)]  per-head 65-col groups, col 64 = 1.0 so
                         the PV matmul also emits the softmax denominator.
  - Scores are emitted as HEAD PAIRS: even head h uses array rows 0-63,
    odd head h+1 rows 64-127 (tile_position auto-derived from base
    partitions) -> the two 64-contraction matmuls run CONCURRENTLY in
    the PE array (row tiling), ~2x scores throughput.
  - attn^T = exp(scores^T/8) on ACT; ACT is the near-bottleneck
    (~147us of exp), so the schedule starts exp as early as possible:
    warmup MMs trip the HAM clock gate + preload the exp table while
    the first K/Q o-tiles stream in; scores for head pair 0 start after
    only 3 projection blocks; all remaining projection blocks are
    interleaved between scores steps to keep the PE busy while ACT
    drains the exp backlog.
  - pv[65, q] = V'_h.T @ attn^T accumulated over 8 k-tiles; raw blocks
    (incl. denominator row 64) are DMA'd out; the final transpose to
    [q, h*64+d] and the divide happen on the HOST.
"""

import numpy as np

B = 8
S = 1024
D = 1024          # D_IN == D_OUT
H = 16
DK = 64           # D_K
KT = 8            # 128-row tiles along a 1024 dim
QC = 2            # q-chunks of 512
P = 128
NCH = 512         # matmul moving free dim
PV_LAG = 3        # pair-iterations between scores emission and PV
N_WARM = 10       # HAM warmup matmuls (~4.3us @ cold clock)

_cache = {}


def _build():
    import concourse.tile as tile
    import concourse.mybir as mybir
    from concourse import bacc

    F32 = mybir.dt.float32
    BF16 = mybir.dt.bfloat16
    Exp = mybir.ActivationFunctionType.Exp
    MMDT = BF16

    nc = bacc.Bacc(None, target_bir_lowering=False, debug=True)

    # host pre-tiles inputs so every DMA chunk is fully contiguous in DRAM
    # (128-256 KB per descriptor -> full per-queue HBM streaming rate)
    xqT = nc.declare_dram_parameter("xqT", [KT, P, S], BF16, isOutput=False)
    xkT = nc.declare_dram_parameter("xkT", [KT, P, S], BF16, isOutput=False)
    xvT = nc.declare_dram_parameter("xvT", [KT, P, S], BF16, isOutput=False)
    wqT = nc.declare_dram_parameter("wqT", [2, KT, P, NCH], BF16, isOutput=False)
    wkT = nc.declare_dram_parameter("wkT", [2, KT, P, NCH], BF16, isOutput=False)
    wvT = nc.declare_dram_parameter("wvT", [2, KT, P, NCH], BF16, isOutput=False)
    bq = nc.declare_dram_parameter("bq", [D], F32, isOutput=False)
    bk = nc.declare_dram_parameter("bk", [D], F32, isOutput=False)
    bv = nc.declare_dram_parameter("bv", [D], F32, isOutput=False)
    # raw per-head PV output + separate softmax denominators;
    # host divides + transposes
    out = nc.declare_dram_parameter("out", [H, DK, S], F32, isOutput=True)
    den = nc.declare_dram_parameter("den", [H, S], F32, isOutput=True)

    with tile.TileContext(nc) as tc:
        with tc.tile_pool(name="persist", bufs=1) as persist, \
             tc.tile_pool(name="attn_p", bufs=2 * (PV_LAG + 1)) as attn_p, \
             tc.tile_pool(name="ot_p", bufs=2) as ot_p, \
             tc.tile_pool(name="scr_p", bufs=2) as scr_p, \
             tc.tile_pool(name="x_p", bufs=3) as x_p, \
             tc.tile_pool(name="w_p", bufs=3) as w_p, \
             tc.tile_pool(name="b_p", bufs=1) as b_p, \
             tc.tile_pool(name="wu_p", bufs=1) as wu_p, \
             tc.tile_pool(name="pp", bufs=2, space="PSUM") as pp, \
             tc.tile_pool(name="sc_p", bufs=2, space="PSUM") as sc_p, \
             tc.tile_pool(name="pv_p", bufs=1, space="PSUM") as pv_p, \
             tc.tile_pool(name="dn_ps", bufs=1, space="PSUM") as dn_ps:

            qT = persist.tile([P, KT, S], MMDT, tag="qT")
            kT = persist.tile([P, KT, S], MMDT, tag="kT")
            vP = persist.tile([P, KT, H * DK], MMDT, tag="vP")

            # ---- input tiles ----
            # Pre-roll loads round-robin sync/gpsimd queues (a single queue
            # sustains only ~170 GB/s, which starved the kernel start);
            # mid-kernel loads stay on sync so the gpsimd queue is free for
            # the bias adds.  NEVER dispatch loads on the scalar queue: a
            # blocked dma_start there stalls the whole exp stream.
            xsrc = {"q": xqT, "k": xkT, "v": xvT}
            wsrc = {"q": wqT, "k": wkT, "v": wvT}
            xtiles, wtiles = {}, {}

            def get_x(kind, eng=None):
                # full-width [P, KT, S] tile, 256 KB contiguous per t-chunk
                if kind not in xtiles:
                    t_ = x_p.tile([P, KT, S], MMDT, tag="x", name=f"x_{kind}")
                    for t in range(KT):
                        (eng or nc.sync).dma_start(
                            out=t_[:, t, :], in_=xsrc[kind][t, :, :])
                    xtiles[kind] = t_
                return xtiles[kind]

            def get_w(kind, oh, eng=None):
                key = (kind, oh)
                if key not in wtiles:
                    t_ = w_p.tile([P, KT, NCH], MMDT, tag="w",
                                  name=f"w_{kind}{oh}")
                    for t in range(KT):
                        (eng or nc.sync).dma_start(
                            out=t_[:, t, :], in_=wsrc[kind][oh, t, :, :])
                    wtiles[key] = t_
                return wtiles[key]

            # prefetch the pre-roll inputs before anything else queues
            get_x("k", eng=nc.sync), get_w("k", 0, eng=nc.gpsimd)
            get_x("q", eng=nc.sync), get_w("q", 0, eng=nc.gpsimd)

            # ---- biases ----
            bqs = b_p.tile([P, KT], F32, tag="bqs")
            bks = b_p.tile([P, KT], F32, tag="bks")
            nc.sync.dma_start(out=bqs[:], in_=bq[:].rearrange("(t p) -> p t", p=P))
            nc.sync.dma_start(out=bks[:], in_=bk[:].rearrange("(t p) -> p t", p=P))
            bvb = b_p.tile([P, D], F32, tag="bvb")
            nc.gpsimd.dma_start(out=bvb[:], in_=bv[:].partition_broadcast(P))

            # all-ones stationary column for the M=1 denominator matmuls
            onesb = b_p.tile([P, 1], MMDT, tag="onesb")
            nc.vector.memset(onesb[:], 1.0)

            # ---- HAM warmup + exp table preload ----
            wu = wu_p.tile([P, NCH], MMDT, tag="wu")
            nc.vector.memset(wu[:], 0.0)
            wuf = wu_p.tile([P, 8], F32, tag="wuf")
            nc.vector.memset(wuf[:], 0.0)
            wue = wu_p.tile([P, 8], F32, tag="wue")
            nc.scalar.activation(out=wue[:], in_=wuf[:], func=Exp, scale=1.0)
            for i in range(N_WARM):
                ps_ = pp.tile([P, NCH], F32, tag="proj", name=f"warm{i}")
                nc.tensor.matmul(ps_[:], wu[:, 0:P], wu[:],
                                 start=True, stop=True)

            # ---- projection blocks ----
            # biases run on the (otherwise idle) gpsimd engine so the proj
            # psum ring never waits behind DVE tree/copy backlog
            def kq_block(kind, ot, sc):
                dst, bias = (qT, bqs) if kind == "q" else (kT, bks)
                w_sb = get_w(kind, ot // 4)
                x_sb = get_x(kind)
                o4 = ot % 4
                ps_ = pp.tile([P, NCH], F32, tag="proj",
                              name=f"ps_{kind}{ot}{sc}")
                for it in range(KT):
                    nc.tensor.matmul(
                        ps_[:],
                        w_sb[:, it, o4 * P:(o4 + 1) * P],
                        x_sb[:, it, sc * NCH:(sc + 1) * NCH],
                        start=(it == 0), stop=(it == KT - 1))
                nc.vector.tensor_scalar_add(
                    out=dst[:, ot, sc * NCH:(sc + 1) * NCH],
                    in0=ps_[:], scalar1=bias[:, ot:ot + 1])

            def v_block(oh, sc, s4):
                w_sb = get_w("v", oh)
                x_sb = get_x("v")
                st = sc * 4 + s4
                ps_ = pp.tile([P, NCH], F32, tag="proj", name=f"ps_v{oh}{st}")
                for it in range(KT):
                    nc.tensor.matmul(
                        ps_[:],
                        x_sb[:, it, sc * NCH + s4 * P:sc * NCH + (s4 + 1) * P],
                        w_sb[:, it, :],
                        start=(it == 0), stop=(it == KT - 1))
                nc.vector.tensor_tensor(
                    out=vP[:, st, oh * NCH:(oh + 1) * NCH],
                    in0=ps_[:],
                    in1=bvb[:, oh * NCH:(oh + 1) * NCH],
                    op=mybir.AluOpType.add)

            def run_block(bid):
                if bid[0] == "v":
                    v_block(bid[1], bid[2], bid[3])
                else:
                    kq_block(bid[0], bid[1], bid[2])
                done_ids.add(bid)

            # pre-roll blocks emitted before the first scores step
            # (scores kb 0-1 only need the first K s-half; K01 is force-
            # drained at kb==2, shortening the startup critical path)
            pre = [("k", 0, 0), ("q", 0, 0)]
            queue = [("k", 0, 1), ("q", 0, 1)]
            for ot in (1,):
                queue += [("k", ot, 0), ("k", ot, 1), ("q", ot, 0), ("q", ot, 1)]
            queue += [("v", 0, sc, s4) for sc in (0, 1) for s4 in range(4)]
            for ot in (2, 3, 4):
                queue += [("k", ot, 0), ("k", ot, 1), ("q", ot, 0), ("q", ot, 1)]
            queue += [("v", 1, sc, s4) for sc in (0, 1) for s4 in range(4)]
            for ot in (5, 6, 7):
                queue += [("k", ot, 0), ("k", ot, 1), ("q", ot, 0), ("q", ot, 1)]

            done_ids = set()
            qpos = [0]

            def drain_n(n):
                took = 0
                while qpos[0] < len(queue) and took < n:
                    run_block(queue[qpos[0]])
                    qpos[0] += 1
                    took += 1

            def drain_through(ids):
                while any(i not in done_ids for i in ids):
                    assert qpos[0] < len(queue), f"missing {ids}"
                    run_block(queue[qpos[0]])
                    qpos[0] += 1

            for b in pre:
                run_block(b)

            # ---- attention emission ----
            iters = [(hp, qc) for hp in range(H // 2) for qc in range(QC)]
            attns = {}

            def sc_step(ap, hp, qc, kb):
                if kb == 0:
                    attns[ap] = (
                        attn_p.tile([P, KT, NCH], MMDT, tag="attnT",
                                    name=f"aA{ap}"),
                        attn_p.tile([P, KT, NCH], MMDT, tag="attnT",
                                    name=f"aB{ap}"),
                    )
                tA, tB = attns[ap]
                At = sc_p.tile([P, 2, NCH], F32, tag="sc", name=f"scA{ap}_{kb}")
                Bt = sc_p.tile([P, 2, NCH], F32, tag="sc", name=f"scB{ap}_{kb}")
                q_lo = qT[0:DK, hp, qc * NCH:(qc + 1) * NCH]
                q_hi = qT[DK:P, hp, qc * NCH:(qc + 1) * NCH]
                for j in (0, 1):
                    kt = 2 * kb + j
                    # even head: array rows 0-63; odd head: rows 64-127.
                    # Emitted adjacently -> the PE runs them concurrently.
                    nc.tensor.matmul(
                        At[:, j, :],
                        kT[0:DK, hp, kt * P:(kt + 1) * P], q_lo,
                        start=True, stop=True)
                    nc.tensor.matmul(
                        Bt[:, j, :],
                        kT[DK:P, hp, kt * P:(kt + 1) * P], q_hi,
                        start=True, stop=True)
                nc.scalar.activation(out=tA[:, 2 * kb:2 * kb + 2, :],
                                     in_=At[:], func=Exp, scale=0.125)
                nc.scalar.activation(out=tB[:, 2 * kb:2 * kb + 2, :],
                                     in_=Bt[:], func=Exp, scale=0.125)

            pending_den = []

            def emit_den():
                """M=1 col-tiled matmuls summing the DVE-reduced attn rows
                -> softmax denominators."""
                ap, sA, sB = pending_den.pop(0)
                hp, qc = iters[ap]
                dn = dn_ps.tile([33, NCH], F32, tag="dn", name=f"dn{ap}")
                nc.tensor.matmul(dn[0:1, :], onesb[:], sA[:, 0, :],
                                 start=True, stop=True)
                nc.tensor.matmul(dn[32:33, :], onesb[:], sB[:, 0, :],
                                 start=True, stop=True)
                dsb = ot_p.tile([33, NCH], F32, tag="dsb", name=f"dsb{ap}")
                nc.vector.tensor_copy(out=dsb[:], in_=dn[:])
                nc.sync.dma_start(
                    out=den[2 * hp, qc * NCH:(qc + 1) * NCH], in_=dsb[0:1, :])
                nc.sync.dma_start(
                    out=den[2 * hp + 1, qc * NCH:(qc + 1) * NCH],
                    in_=dsb[32:33, :])

            def tree(t_, name):
                """kt-reduction of attnT [P, KT, NCH] -> scratch[:, 0, :].
                Writes scratch (not attnT) so the attnT slot is freed by the
                PV matmuls.  The big level-1 add runs on the otherwise-idle
                gpsimd engine (SBUF-only there); levels 2-3 on DVE."""
                s_ = scr_p.tile([P, 4, NCH], MMDT, tag="scr", name=name)
                nc.gpsimd.tensor_tensor(out=s_[:], in0=t_[:, 0:4, :],
                                        in1=t_[:, 4:8, :],
                                        op=mybir.AluOpType.add)
                nc.vector.tensor_tensor(out=s_[:, 0:2, :], in0=s_[:, 0:2, :],
                                        in1=s_[:, 2:4, :],
                                        op=mybir.AluOpType.add)
                nc.vector.tensor_tensor(out=s_[:, 0:1, :], in0=s_[:, 0:1, :],
                                        in1=s_[:, 1:2, :],
                                        op=mybir.AluOpType.add)
                return s_

            def pv_pair(ap):
                hp, qc = iters[ap]
                oh = hp // 4
                drain_through([("v", oh, sc, s4)
                               for sc in (0, 1) for s4 in range(4)])
                tA, tB = attns.pop(ap)
                hA, hB = 2 * hp, 2 * hp + 1
                # col-tiled pair: head A -> array cols 0-63 / psum parts 0-63,
                # head B -> cols 64-127; interleaved so they run concurrently
                pv = pv_p.tile([P, NCH], F32, tag="pv", name=f"pv{ap}")
                for kt in range(KT):
                    nc.tensor.matmul(
                        pv[0:DK, :],
                        vP[:, kt, hA * DK:(hA + 1) * DK], tA[:, kt, :],
                        start=(kt == 0), stop=(kt == KT - 1),
                        skip_group_check=True)
                    nc.tensor.matmul(
                        pv[DK:P, :],
                        vP[:, kt, hB * DK:(hB + 1) * DK], tB[:, kt, :],
                        start=(kt == 0), stop=(kt == KT - 1),
                        skip_group_check=True)
                ot_sb = ot_p.tile([P, NCH], F32, tag="ot", name=f"ot{ap}")
                nc.vector.tensor_copy(out=ot_sb[:], in_=pv[:])
                nc.sync.dma_start(
                    out=out[hA, :, qc * NCH:(qc + 1) * NCH], in_=ot_sb[0:DK, :])
                nc.sync.dma_start(
                    out=out[hB, :, qc * NCH:(qc + 1) * NCH], in_=ot_sb[DK:P, :])
                sA = tree(tA, f"sA{ap}")
                sB = tree(tB, f"sB{ap}")
                pending_den.append((ap, sA, sB))

            pvi = [0]

            def maybe_pv():
                if pvi[0] < len(iters):
                    pv_pair(pvi[0])
                    pvi[0] += 1

            for ap, (hp, qc) in enumerate(iters):
                if pending_den:
                    emit_den()
                for kb in range(KT // 2):
                    if kb == 0:
                        drain_through([("k", hp, 0), ("q", hp, qc)])
                    elif kb == 2:
                        drain_through([("k", hp, 1)])
                    sc_step(ap, hp, qc, kb)
                    if kb < 3:
                        drain_n(1)
                    if kb == 1 and ap >= PV_LAG:
                        # mid-iter PV: the trees it queues clear the DVE well
                        # before the next iter's first exp WARs their inputs
                        maybe_pv()
                if ap >= len(iters) - 2:   # shrink the end-of-kernel PV tail
                    maybe_pv()
            drain_n(len(queue))
            while pvi[0] < len(iters):
                maybe_pv()
            while pending_den:
                emit_den()
            assert qpos[0] == len(queue) and not attns

    nc.finalize()
    return nc


def _get_program():
    key = "prog"
    if key not in _cache:
        _cache[key] = _build()
    return _cache[key]


def _prep_in_maps(inputs):
    import ml_dtypes

    BF = ml_dtypes.bfloat16

    def prep_w(W):
        # W [D_OUT, D_IN] -> W.T [D_IN, D_OUT] -> [oh, t, p, s] DMA-native
        wT = np.asarray(W, dtype=np.float32).T.astype(BF)
        return np.ascontiguousarray(
            wT.reshape(KT, P, 2, NCH).transpose(2, 0, 1, 3))

    def prep_x(x):
        # x [S, D_IN] -> x.T [D_IN, S] -> [t, p, s] DMA-native
        return np.ascontiguousarray(
            np.asarray(x, dtype=np.float32).T.astype(BF).reshape(KT, P, S))

    query = np.asarray(inputs["query"], dtype=np.float32)
    key_ = np.asarray(inputs["key_"], dtype=np.float32)
    value = np.asarray(inputs["value"], dtype=np.float32)
    wqT, wkT, wvT = (prep_w(inputs[k]) for k in ("Wq", "Wk", "Wv"))
    bq = np.ascontiguousarray(np.asarray(inputs["bq"], dtype=np.float32))
    bk = np.ascontiguousarray(np.asarray(inputs["bk"], dtype=np.float32))
    bv = np.ascontiguousarray(np.asarray(inputs["bv"], dtype=np.float32))
    return [
        {
            "xqT": prep_x(query[b]),
            "xkT": prep_x(key_[b]),
            "xvT": prep_x(value[b]),
            "wqT": wqT, "wkT": wkT, "wvT": wvT,
            "bq": bq, "bk": bk, "bv": bv,
        }
        for b in range(B)
    ]


def kernel(query, key_, value, Wq, bq, Wk, bk, Wv, bv):
    from concourse.bass_utils import run_bass_kernel_spmd

    nc = _get_program()
    in_maps = _prep_in_maps(dict(
        query=query, key_=key_, value=value,
        Wq=Wq, bq=bq, Wk=Wk, bk=bk, Wv=Wv, bv=bv,
    ))
    res = run_bass_kernel_spmd(nc, in_maps, list(range(B)))
    out = np.empty((B, S * D), dtype=np.float32)
    for b in range(B):
        o = np.asarray(res.results[b]["out"])          # [H, DK, S]
        dn = np.asarray(res.results[b]["den"])         # [H, S]
        x = o / dn[:, None, :]                         # [H, DK, S]
        out[b] = x.transpose(2, 0, 1).reshape(-1)      # [S, H*DK] flattened
    return out


# revision 34
# speedup vs baseline: 1.2071x; 1.2071x over previous
"""Multi-head attention kernel for Trainium2, 8 NeuronCores, data-parallel over batch.

Problem (matches the reference nn.Module):
  B=8, S=1024, D_IN=D_OUT=1024, H=16, D_K=64, fp32 in/out.
  q/k/v = Linear(x) per input; scores = q k^T / sqrt(64); attn = softmax;
  out = (attn v) heads-concatenated -> [B, S*D_OUT].

Strategy:
  - One batch element per core (8 cores). No collectives.
  - All matmul operands bf16 (PSUM f32). Host pre-transposes inputs.
  - On-chip layouts:
      kT/qT [P, KT, S]  (o = p + 128*t; head h lives at t=h//2,
                         partitions (h%2)*64..+64)
      vP    [P, KT, H*(DK+1)]  per-head 65-col groups, col 64 = 1.0 so
                         the PV matmul also emits the softmax denominator.
  - Scores are emitted as HEAD PAIRS: even head h uses array rows 0-63,
    odd head h+1 rows 64-127 (tile_position auto-derived from base
    partitions) -> the two 64-contraction matmuls run CONCURRENTLY in
    the PE array (row tiling), ~2x scores throughput.
  - attn^T = exp(scores^T/8) on ACT; ACT is the near-bottleneck
    (~147us of exp), so the schedule starts exp as early as possible:
    warmup MMs trip the HAM clock gate + preload the exp table while
    the first K/Q o-tiles stream in; scores for head pair 0 start after
    only 3 projection blocks; all remaining projection blocks are
    interleaved between scores steps to keep the PE busy while ACT
    drains the exp backlog.
  - pv[65, q] = V'_h.T @ attn^T accumulated over 8 k-tiles; raw blocks
    (incl. denominator row 64) are DMA'd out; the final transpose to
    [q, h*64+d] and the divide happen on the HOST.
"""

import numpy as np

B = 8
S = 1024
D = 1024          # D_IN == D_OUT
H = 16
DK = 64           # D_K
KT = 8            # 128-row tiles along a 1024 dim
QC = 2            # q-chunks of 512
P = 128
NCH = 512         # matmul moving free dim
PV_LAG = 3        # pair-iterations between scores emission and PV
N_WARM = 10       # HAM warmup matmuls (~4.3us @ cold clock)

_cache = {}


def _build():
    import concourse.tile as tile
    import concourse.mybir as mybir
    from concourse import bacc

    F32 = mybir.dt.float32
    BF16 = mybir.dt.bfloat16
    Exp = mybir.ActivationFunctionType.Exp
    MMDT = BF16

    nc = bacc.Bacc(None, target_bir_lowering=False, debug=True)

    # host pre-tiles inputs so every DMA chunk is fully contiguous in DRAM
    # (128-256 KB per descriptor -> full per-queue HBM streaming rate)
    xqT = nc.declare_dram_parameter("xqT", [KT, P, S], BF16, isOutput=False)
    xkT = nc.declare_dram_parameter("xkT", [KT, P, S], BF16, isOutput=False)
    xvT = nc.declare_dram_parameter("xvT", [KT, P, S], BF16, isOutput=False)
    wqT = nc.declare_dram_parameter("wqT", [2, KT, P, NCH], BF16, isOutput=False)
    wkT = nc.declare_dram_parameter("wkT", [2, KT, P, NCH], BF16, isOutput=False)
    wvT = nc.declare_dram_parameter("wvT", [2, KT, P, NCH], BF16, isOutput=False)
    bq = nc.declare_dram_parameter("bq", [D], F32, isOutput=False)
    bk = nc.declare_dram_parameter("bk", [D], F32, isOutput=False)
    bv = nc.declare_dram_parameter("bv", [D], F32, isOutput=False)
    # raw per-head PV output incl. denominator row 64 (from the all-ones
    # V' column); host divides + transposes
    out = nc.declare_dram_parameter("out", [H, DK + 1, S], F32, isOutput=True)

    with tile.TileContext(nc) as tc:
        with tc.tile_pool(name="persist", bufs=1) as persist, \
             tc.tile_pool(name="attn_p", bufs=2 * (PV_LAG + 1)) as attn_p, \
             tc.tile_pool(name="ot_p", bufs=3) as ot_p, \
             tc.tile_pool(name="x_p", bufs=3) as x_p, \
             tc.tile_pool(name="w_p", bufs=3) as w_p, \
             tc.tile_pool(name="b_p", bufs=1) as b_p, \
             tc.tile_pool(name="wu_p", bufs=1) as wu_p, \
             tc.tile_pool(name="pp", bufs=2, space="PSUM") as pp, \
             tc.tile_pool(name="sc_p", bufs=2, space="PSUM") as sc_p, \
             tc.tile_pool(name="pv_p", bufs=2, space="PSUM") as pv_p:

            qT = persist.tile([P, KT, S], MMDT, tag="qT")
            kT = persist.tile([P, KT, S], MMDT, tag="kT")
            vP = persist.tile([P, KT, H * (DK + 1)], MMDT, tag="vP")

            # ---- input tiles ----
            # Pre-roll loads round-robin sync/gpsimd queues (a single queue
            # sustains only ~170 GB/s, which starved the kernel start);
            # mid-kernel loads stay on sync so the gpsimd queue is free for
            # the bias adds.  NEVER dispatch loads on the scalar queue: a
            # blocked dma_start there stalls the whole exp stream.
            xsrc = {"q": xqT, "k": xkT, "v": xvT}
            wsrc = {"q": wqT, "k": wkT, "v": wvT}
            xtiles, wtiles = {}, {}

            def get_x(kind, eng=None):
                # full-width [P, KT, S] tile, 256 KB contiguous per t-chunk
                if kind not in xtiles:
                    t_ = x_p.tile([P, KT, S], MMDT, tag="x", name=f"x_{kind}")
                    for t in range(KT):
                        (eng or nc.sync).dma_start(
                            out=t_[:, t, :], in_=xsrc[kind][t, :, :])
                    xtiles[kind] = t_
                return xtiles[kind]

            def get_w(kind, oh, eng=None):
                key = (kind, oh)
                if key not in wtiles:
                    t_ = w_p.tile([P, KT, NCH], MMDT, tag="w",
                                  name=f"w_{kind}{oh}")
                    for t in range(KT):
                        (eng or nc.sync).dma_start(
                            out=t_[:, t, :], in_=wsrc[kind][oh, t, :, :])
                    wtiles[key] = t_
                return wtiles[key]

            # prefetch the pre-roll inputs before anything else queues
            get_x("k", eng=nc.sync), get_w("k", 0, eng=nc.gpsimd)
            get_x("q", eng=nc.sync), get_w("q", 0, eng=nc.gpsimd)

            # ---- biases ----
            bqs = b_p.tile([P, KT], F32, tag="bqs")
            bks = b_p.tile([P, KT], F32, tag="bks")
            nc.sync.dma_start(out=bqs[:], in_=bq[:].rearrange("(t p) -> p t", p=P))
            nc.sync.dma_start(out=bks[:], in_=bk[:].rearrange("(t p) -> p t", p=P))
            bvb = b_p.tile([P, D], F32, tag="bvb")
            nc.gpsimd.dma_start(out=bvb[:], in_=bv[:].partition_broadcast(P))

            # ones columns of V' (V-block writes skip col 64 of each group)
            ones16 = b_p.tile([P, H], F32, tag="ones16")
            nc.vector.memset(ones16[:], 1.0)
            for st in range(KT):
                nc.vector.tensor_copy(
                    out=vP[:, st, :]
                    .rearrange("p (h d) -> p h d", h=H)[:, :, DK:DK + 1],
                    in_=ones16[:].unsqueeze(2),
                )

            # ---- HAM warmup + exp table preload ----
            wu = wu_p.tile([P, NCH], MMDT, tag="wu")
            nc.vector.memset(wu[:], 0.0)
            wuf = wu_p.tile([P, 8], F32, tag="wuf")
            nc.vector.memset(wuf[:], 0.0)
            wue = wu_p.tile([P, 8], F32, tag="wue")
            nc.scalar.activation(out=wue[:], in_=wuf[:], func=Exp, scale=1.0)
            for i in range(N_WARM):
                ps_ = pp.tile([P, NCH], F32, tag="proj", name=f"warm{i}")
                nc.tensor.matmul(ps_[:], wu[:, 0:P], wu[:],
                                 start=True, stop=True)

            # ---- projection blocks ----
            # biases run on the (otherwise idle) gpsimd engine so the proj
            # psum ring never waits behind DVE tree/copy backlog
            def kq_block(kind, ot, sc):
                dst, bias = (qT, bqs) if kind == "q" else (kT, bks)
                w_sb = get_w(kind, ot // 4)
                x_sb = get_x(kind)
                o4 = ot % 4
                ps_ = pp.tile([P, NCH], F32, tag="proj",
                              name=f"ps_{kind}{ot}{sc}")
                for it in range(KT):
                    nc.tensor.matmul(
                        ps_[:],
                        w_sb[:, it, o4 * P:(o4 + 1) * P],
                        x_sb[:, it, sc * NCH:(sc + 1) * NCH],
                        start=(it == 0), stop=(it == KT - 1))
                nc.vector.tensor_scalar_add(
                    out=dst[:, ot, sc * NCH:(sc + 1) * NCH],
                    in0=ps_[:], scalar1=bias[:, ot:ot + 1])

            def v_block(oh, sc, s4):
                w_sb = get_w("v", oh)
                x_sb = get_x("v")
                st = sc * 4 + s4
                ps_ = pp.tile([P, NCH], F32, tag="proj", name=f"ps_v{oh}{st}")
                for it in range(KT):
                    nc.tensor.matmul(
                        ps_[:],
                        x_sb[:, it, sc * NCH + s4 * P:sc * NCH + (s4 + 1) * P],
                        w_sb[:, it, :],
                        start=(it == 0), stop=(it == KT - 1))
                nc.vector.tensor_tensor(
                    out=vP[:, st, :]
                    .rearrange("p (h d) -> p h d", h=H)
                    [:, oh * 8:(oh + 1) * 8, 0:DK],
                    in0=ps_[:].rearrange("p (h d) -> p h d", h=8),
                    in1=bvb[:, oh * NCH:(oh + 1) * NCH]
                    .rearrange("p (h d) -> p h d", h=8),
                    op=mybir.AluOpType.add)

            def run_block(bid):
                if bid[0] == "v":
                    v_block(bid[1], bid[2], bid[3])
                else:
                    kq_block(bid[0], bid[1], bid[2])
                done_ids.add(bid)

            # pre-roll blocks emitted before the first scores step
            # (scores kb 0-1 only need the first K s-half; K01 is force-
            # drained at kb==2, shortening the startup critical path)
            pre = [("k", 0, 0), ("q", 0, 0)]
            queue = [("k", 0, 1), ("q", 0, 1)]
            for ot in (1,):
                queue += [("k", ot, 0), ("k", ot, 1), ("q", ot, 0), ("q", ot, 1)]
            queue += [("v", 0, sc, s4) for sc in (0, 1) for s4 in range(4)]
            for ot in (2, 3, 4):
                queue += [("k", ot, 0), ("k", ot, 1), ("q", ot, 0), ("q", ot, 1)]
            queue += [("v", 1, sc, s4) for sc in (0, 1) for s4 in range(4)]
            for ot in (5, 6, 7):
                queue += [("k", ot, 0), ("k", ot, 1), ("q", ot, 0), ("q", ot, 1)]

            done_ids = set()
            qpos = [0]

            def drain_n(n):
                took = 0
                while qpos[0] < len(queue) and took < n:
                    run_block(queue[qpos[0]])
                    qpos[0] += 1
                    took += 1

            def drain_through(ids):
                while any(i not in done_ids for i in ids):
                    assert qpos[0] < len(queue), f"missing {ids}"
                    run_block(queue[qpos[0]])
                    qpos[0] += 1

            for b in pre:
                run_block(b)

            # ---- attention emission ----
            iters = [(hp, qc) for hp in range(H // 2) for qc in range(QC)]
            attns = {}

            def sc_step(ap, hp, qc, kb):
                if kb == 0:
                    attns[ap] = (
                        attn_p.tile([P, KT, NCH], MMDT, tag="attnT",
                                    name=f"aA{ap}"),
                        attn_p.tile([P, KT, NCH], MMDT, tag="attnT",
                                    name=f"aB{ap}"),
                    )
                tA, tB = attns[ap]
                At = sc_p.tile([P, 2, NCH], F32, tag="sc", name=f"scA{ap}_{kb}")
                Bt = sc_p.tile([P, 2, NCH], F32, tag="sc", name=f"scB{ap}_{kb}")
                q_lo = qT[0:DK, hp, qc * NCH:(qc + 1) * NCH]
                q_hi = qT[DK:P, hp, qc * NCH:(qc + 1) * NCH]
                for j in (0, 1):
                    kt = 2 * kb + j
                    # even head: array rows 0-63; odd head: rows 64-127.
                    # Emitted adjacently -> the PE runs them concurrently.
                    nc.tensor.matmul(
                        At[:, j, :],
                        kT[0:DK, hp, kt * P:(kt + 1) * P], q_lo,
                        start=True, stop=True)
                    nc.tensor.matmul(
                        Bt[:, j, :],
                        kT[DK:P, hp, kt * P:(kt + 1) * P], q_hi,
                        start=True, stop=True)
                nc.scalar.activation(out=tA[:, 2 * kb:2 * kb + 2, :],
                                     in_=At[:], func=Exp, scale=0.125)
                nc.scalar.activation(out=tB[:, 2 * kb:2 * kb + 2, :],
                                     in_=Bt[:], func=Exp, scale=0.125)

            def emit_out(qc, h, attnT, ap):
                """PV matmul (M=65 incl. the ones/denominator column),
                copy to SBUF, DMA the raw block out."""
                pv = pv_p.tile([DK + 1, NCH], F32, tag="pv",
                               name=f"pv{ap}_{h}")
                for kt in range(KT):
                    nc.tensor.matmul(
                        pv[:],
                        vP[:, kt, h * (DK + 1):(h + 1) * (DK + 1)],
                        attnT[:, kt, :],
                        start=(kt == 0), stop=(kt == KT - 1))
                ot_sb = ot_p.tile([DK + 1, NCH], F32, tag="ot",
                                  name=f"ot{ap}_{h}")
                nc.vector.tensor_copy(out=ot_sb[:], in_=pv[:])
                nc.sync.dma_start(
                    out=out[h, :, qc * NCH:(qc + 1) * NCH], in_=ot_sb[:])

            def pv_pair(ap):
                hp, qc = iters[ap]
                oh = hp // 4
                drain_through([("v", oh, sc, s4)
                               for sc in (0, 1) for s4 in range(4)])
                tA, tB = attns.pop(ap)
                emit_out(qc, 2 * hp, tA, ap)
                emit_out(qc, 2 * hp + 1, tB, ap)

            pvi = [0]

            def maybe_pv():
                if pvi[0] < len(iters):
                    pv_pair(pvi[0])
                    pvi[0] += 1

            for ap, (hp, qc) in enumerate(iters):
                for kb in range(KT // 2):
                    if kb == 0:
                        drain_through([("k", hp, 0), ("q", hp, qc)])
                    elif kb == 2:
                        drain_through([("k", hp, 1)])
                    sc_step(ap, hp, qc, kb)
                    if kb < 3:
                        drain_n(1)
                    if kb == 1 and ap >= PV_LAG:
                        maybe_pv()   # mid-iter PV spreads the PE load
                if ap >= len(iters) - 2:   # shrink the end-of-kernel PV tail
                    maybe_pv()
            drain_n(len(queue))
            while pvi[0] < len(iters):
                maybe_pv()
            assert qpos[0] == len(queue) and not attns

    nc.finalize()
    return nc


def _get_program():
    key = "prog"
    if key not in _cache:
        _cache[key] = _build()
    return _cache[key]


def _prep_in_maps(inputs):
    import ml_dtypes

    BF = ml_dtypes.bfloat16

    def prep_w(W):
        # W [D_OUT, D_IN] -> W.T [D_IN, D_OUT] -> [oh, t, p, s] DMA-native
        wT = np.asarray(W, dtype=np.float32).T.astype(BF)
        return np.ascontiguousarray(
            wT.reshape(KT, P, 2, NCH).transpose(2, 0, 1, 3))

    def prep_x(x):
        # x [S, D_IN] -> x.T [D_IN, S] -> [t, p, s] DMA-native
        return np.ascontiguousarray(
            np.asarray(x, dtype=np.float32).T.astype(BF).reshape(KT, P, S))

    query = np.asarray(inputs["query"], dtype=np.float32)
    key_ = np.asarray(inputs["key_"], dtype=np.float32)
    value = np.asarray(inputs["value"], dtype=np.float32)
    wqT, wkT, wvT = (prep_w(inputs[k]) for k in ("Wq", "Wk", "Wv"))
    bq = np.ascontiguousarray(np.asarray(inputs["bq"], dtype=np.float32))
    bk = np.ascontiguousarray(np.asarray(inputs["bk"], dtype=np.float32))
    bv = np.ascontiguousarray(np.asarray(inputs["bv"], dtype=np.float32))
    return [
        {
            "xqT": prep_x(query[b]),
            "xkT": prep_x(key_[b]),
            "xvT": prep_x(value[b]),
            "wqT": wqT, "wkT": wkT, "wvT": wvT,
            "bq": bq, "bk": bk, "bv": bv,
        }
        for b in range(B)
    ]


def kernel(query, key_, value, Wq, bq, Wk, bk, Wv, bv):
    from concourse.bass_utils import run_bass_kernel_spmd

    nc = _get_program()
    in_maps = _prep_in_maps(dict(
        query=query, key_=key_, value=value,
        Wq=Wq, bq=bq, Wk=Wk, bk=bk, Wv=Wv, bv=bv,
    ))
    res = run_bass_kernel_spmd(nc, in_maps, list(range(B)))
    out = np.empty((B, S * D), dtype=np.float32)
    for b in range(B):
        o = np.asarray(res.results[b]["out"])          # [H, DK+1, S]
        x = o[:, :DK, :] / o[:, DK:DK + 1, :]          # [H, DK, S]
        out[b] = x.transpose(2, 0, 1).reshape(-1)      # [S, H*DK] flattened
    return out


# revision 38
# speedup vs baseline: 1.3127x; 1.0875x over previous
"""Multi-head attention kernel for Trainium2, 8 NeuronCores, data-parallel over batch.

Problem (matches the reference nn.Module):
  B=8, S=1024, D_IN=D_OUT=1024, H=16, D_K=64, fp32 in/out.
  q/k/v = Linear(x) per input; scores = q k^T / sqrt(64); attn = softmax;
  out = (attn v) heads-concatenated -> [B, S*D_OUT].

Strategy:
  - One batch element per core (8 cores). No collectives.
  - All matmul operands bf16 (PSUM f32). Host pre-transposes inputs.
  - On-chip layouts:
      kT/qT [P, KT, S]  (o = p + 128*t; head h lives at t=h//2,
                         partitions (h%2)*64..+64)
      vP    [P, KT, H*(DK+1)]  per-head 65-col groups, col 64 = 1.0 so
                         the PV matmul also emits the softmax denominator.
  - Scores are emitted as HEAD PAIRS: even head h uses array rows 0-63,
    odd head h+1 rows 64-127 (tile_position auto-derived from base
    partitions) -> the two 64-contraction matmuls run CONCURRENTLY in
    the PE array (row tiling), ~2x scores throughput.
  - attn^T = exp(scores^T/8) on ACT; ACT is the near-bottleneck
    (~147us of exp), so the schedule starts exp as early as possible:
    warmup MMs trip the HAM clock gate + preload the exp table while
    the first K/Q o-tiles stream in; scores for head pair 0 start after
    only 3 projection blocks; all remaining projection blocks are
    interleaved between scores steps to keep the PE busy while ACT
    drains the exp backlog.
  - pv[65, q] = V'_h.T @ attn^T accumulated over 8 k-tiles; raw blocks
    (incl. denominator row 64) are DMA'd out; the final transpose to
    [q, h*64+d] and the divide happen on the HOST.
"""

import numpy as np

B = 8
S = 1024
D = 1024          # D_IN == D_OUT
H = 16
DK = 64           # D_K
KT = 8            # 128-row tiles along a 1024 dim
QC = 2            # q-chunks of 512
P = 128
NCH = 512         # matmul moving free dim
PV_LAG = 3        # pair-iterations between scores emission and PV
N_WARM = 10       # HAM warmup matmuls (~4.3us @ cold clock)

_cache = {}


def _build():
    import concourse.tile as tile
    import concourse.mybir as mybir
    from concourse import bacc

    F32 = mybir.dt.float32
    BF16 = mybir.dt.bfloat16
    Exp = mybir.ActivationFunctionType.Exp
    MMDT = BF16

    nc = bacc.Bacc(None, target_bir_lowering=False, debug=True)

    # host pre-tiles inputs so every DMA chunk is fully contiguous in DRAM
    # (128-256 KB per descriptor -> full per-queue HBM streaming rate)
    xqT = nc.declare_dram_parameter("xqT", [KT, P, S], BF16, isOutput=False)
    xkT = nc.declare_dram_parameter("xkT", [KT, P, S], BF16, isOutput=False)
    xvT = nc.declare_dram_parameter("xvT", [KT, P, S], BF16, isOutput=False)
    wqT = nc.declare_dram_parameter("wqT", [2, KT, P, NCH], BF16, isOutput=False)
    wkT = nc.declare_dram_parameter("wkT", [2, KT, P, NCH], BF16, isOutput=False)
    wvT = nc.declare_dram_parameter("wvT", [2, KT, P, NCH], BF16, isOutput=False)
    bq = nc.declare_dram_parameter("bq", [D], F32, isOutput=False)
    bk = nc.declare_dram_parameter("bk", [D], F32, isOutput=False)
    bv = nc.declare_dram_parameter("bv", [D], F32, isOutput=False)
    # raw per-head PV output incl. denominator row 64 (from the all-ones
    # V' column); host divides + transposes
    out = nc.declare_dram_parameter("out", [H, DK + 1, S], F32, isOutput=True)

    with tile.TileContext(nc) as tc:
        with tc.tile_pool(name="persist", bufs=1) as persist, \
             tc.tile_pool(name="attn_p", bufs=PV_LAG + 1) as attn_p, \
             tc.tile_pool(name="ot_p", bufs=3) as ot_p, \
             tc.tile_pool(name="x_p", bufs=3) as x_p, \
             tc.tile_pool(name="w_p", bufs=3) as w_p, \
             tc.tile_pool(name="b_p", bufs=1) as b_p, \
             tc.tile_pool(name="wu_p", bufs=1) as wu_p, \
             tc.tile_pool(name="pp", bufs=2, space="PSUM") as pp, \
             tc.tile_pool(name="sc_p", bufs=2, space="PSUM") as sc_p, \
             tc.tile_pool(name="pv_p", bufs=2, space="PSUM") as pv_p:

            qT = persist.tile([P, KT, S], MMDT, tag="qT")
            kT = persist.tile([P, KT, S], MMDT, tag="kT")
            vP = persist.tile([P, KT, H * (DK + 1)], MMDT, tag="vP")

            # ---- input tiles ----
            # Pre-roll loads round-robin sync/gpsimd queues (a single queue
            # sustains only ~170 GB/s, which starved the kernel start);
            # mid-kernel loads stay on sync so the gpsimd queue is free for
            # the bias adds.  NEVER dispatch loads on the scalar queue: a
            # blocked dma_start there stalls the whole exp stream.
            xsrc = {"q": xqT, "k": xkT, "v": xvT}
            wsrc = {"q": wqT, "k": wkT, "v": wvT}
            xtiles, wtiles = {}, {}

            def get_x(kind, eng=None):
                # full-width [P, KT, S] tile, 256 KB contiguous per t-chunk
                if kind not in xtiles:
                    t_ = x_p.tile([P, KT, S], MMDT, tag="x", name=f"x_{kind}")
                    for t in range(KT):
                        (eng or nc.sync).dma_start(
                            out=t_[:, t, :], in_=xsrc[kind][t, :, :])
                    xtiles[kind] = t_
                return xtiles[kind]

            def get_w(kind, oh, eng=None):
                key = (kind, oh)
                if key not in wtiles:
                    t_ = w_p.tile([P, KT, NCH], MMDT, tag="w",
                                  name=f"w_{kind}{oh}")
                    for t in range(KT):
                        (eng or nc.sync).dma_start(
                            out=t_[:, t, :], in_=wsrc[kind][oh, t, :, :])
                    wtiles[key] = t_
                return wtiles[key]

            # prefetch the pre-roll inputs before anything else queues
            get_x("k", eng=nc.sync), get_w("k", 0, eng=nc.gpsimd)
            get_x("q", eng=nc.sync), get_w("q", 0, eng=nc.gpsimd)

            # ---- biases ----
            bqs = b_p.tile([P, KT], F32, tag="bqs")
            bks = b_p.tile([P, KT], F32, tag="bks")
            nc.sync.dma_start(out=bqs[:], in_=bq[:].rearrange("(t p) -> p t", p=P))
            nc.sync.dma_start(out=bks[:], in_=bk[:].rearrange("(t p) -> p t", p=P))
            bvb = b_p.tile([P, D], F32, tag="bvb")
            nc.gpsimd.dma_start(out=bvb[:], in_=bv[:].partition_broadcast(P))

            # ones columns of V' (V-block writes skip col 64 of each group)
            ones16 = b_p.tile([P, H], F32, tag="ones16")
            nc.vector.memset(ones16[:], 1.0)
            for st in range(KT):
                nc.vector.tensor_copy(
                    out=vP[:, st, :]
                    .rearrange("p (h d) -> p h d", h=H)[:, :, DK:DK + 1],
                    in_=ones16[:].unsqueeze(2),
                )

            # ---- HAM warmup + exp table preload ----
            wu = wu_p.tile([P, NCH], MMDT, tag="wu")
            nc.vector.memset(wu[:], 0.0)
            wuf = wu_p.tile([P, 8], F32, tag="wuf")
            nc.vector.memset(wuf[:], 0.0)
            wue = wu_p.tile([P, 8], F32, tag="wue")
            nc.scalar.activation(out=wue[:], in_=wuf[:], func=Exp, scale=1.0)
            for i in range(N_WARM):
                ps_ = pp.tile([P, NCH], F32, tag="proj", name=f"warm{i}")
                nc.tensor.matmul(ps_[:], wu[:, 0:P], wu[:],
                                 start=True, stop=True)

            # ---- projection blocks ----
            # biases run on the (otherwise idle) gpsimd engine so the proj
            # psum ring never waits behind DVE tree/copy backlog
            def kq_block(kind, ot, sc):
                dst, bias = (qT, bqs) if kind == "q" else (kT, bks)
                w_sb = get_w(kind, ot // 4)
                x_sb = get_x(kind)
                o4 = ot % 4
                ps_ = pp.tile([P, NCH], F32, tag="proj",
                              name=f"ps_{kind}{ot}{sc}")
                for it in range(KT):
                    nc.tensor.matmul(
                        ps_[:],
                        w_sb[:, it, o4 * P:(o4 + 1) * P],
                        x_sb[:, it, sc * NCH:(sc + 1) * NCH],
                        start=(it == 0), stop=(it == KT - 1))
                nc.vector.tensor_scalar_add(
                    out=dst[:, ot, sc * NCH:(sc + 1) * NCH],
                    in0=ps_[:], scalar1=bias[:, ot:ot + 1])

            def v_block(oh, sc, s4):
                w_sb = get_w("v", oh)
                x_sb = get_x("v")
                st = sc * 4 + s4
                ps_ = pp.tile([P, NCH], F32, tag="proj", name=f"ps_v{oh}{st}")
                for it in range(KT):
                    nc.tensor.matmul(
                        ps_[:],
                        x_sb[:, it, sc * NCH + s4 * P:sc * NCH + (s4 + 1) * P],
                        w_sb[:, it, :],
                        start=(it == 0), stop=(it == KT - 1))
                nc.vector.tensor_tensor(
                    out=vP[:, st, :]
                    .rearrange("p (h d) -> p h d", h=H)
                    [:, oh * 8:(oh + 1) * 8, 0:DK],
                    in0=ps_[:].rearrange("p (h d) -> p h d", h=8),
                    in1=bvb[:, oh * NCH:(oh + 1) * NCH]
                    .rearrange("p (h d) -> p h d", h=8),
                    op=mybir.AluOpType.add)

            def run_block(bid):
                if bid[0] == "v":
                    v_block(bid[1], bid[2], bid[3])
                else:
                    kq_block(bid[0], bid[1], bid[2])
                done_ids.add(bid)

            # pre-roll blocks emitted before the first scores step
            # (scores kb 0-1 only need the first K s-half; K01 is force-
            # drained at kb==2, shortening the startup critical path)
            pre = [("k", 0, 0), ("q", 0, 0)]
            queue = [("k", 0, 1), ("q", 0, 1)]
            for ot in (1,):
                queue += [("k", ot, 0), ("k", ot, 1), ("q", ot, 0), ("q", ot, 1)]
            queue += [("v", 0, sc, s4) for sc in (0, 1) for s4 in range(4)]
            for ot in (2, 3, 4):
                queue += [("k", ot, 0), ("k", ot, 1), ("q", ot, 0), ("q", ot, 1)]
            queue += [("v", 1, sc, s4) for sc in (0, 1) for s4 in range(4)]
            for ot in (5, 6, 7):
                queue += [("k", ot, 0), ("k", ot, 1), ("q", ot, 0), ("q", ot, 1)]

            done_ids = set()
            qpos = [0]

            def drain_n(n):
                took = 0
                while qpos[0] < len(queue) and took < n:
                    run_block(queue[qpos[0]])
                    qpos[0] += 1
                    took += 1

            def drain_through(ids):
                while any(i not in done_ids for i in ids):
                    assert qpos[0] < len(queue), f"missing {ids}"
                    run_block(queue[qpos[0]])
                    qpos[0] += 1

            for b in pre:
                run_block(b)

            # ---- attention emission ----
            iters = [(hp, qc) for hp in range(H // 2) for qc in range(QC)]
            attns = {}

            def sc_step(ap, hp, qc, kt):
                """One k-tile of scores for BOTH heads of the pair.  Both
                matmuls write ONE psum tile and one exp call consumes it, so
                their WAR readiness is simultaneous -> the Tile scheduler
                issues them adjacently -> the PE runs them concurrently in
                disjoint row groups (even head rows 0-63, odd head 64-127)."""
                if kt == 0:
                    attns[ap] = attn_p.tile([P, KT, 2, NCH], MMDT,
                                            tag="attnT", name=f"a{ap}")
                tP = attns[ap]
                sc_t = sc_p.tile([P, 2, NCH], F32, tag="sc",
                                 name=f"sc{ap}_{kt}")
                nc.tensor.matmul(
                    sc_t[:, 0, :],
                    kT[0:DK, hp, kt * P:(kt + 1) * P],
                    qT[0:DK, hp, qc * NCH:(qc + 1) * NCH],
                    start=True, stop=True)
                nc.tensor.matmul(
                    sc_t[:, 1, :],
                    kT[DK:P, hp, kt * P:(kt + 1) * P],
                    qT[DK:P, hp, qc * NCH:(qc + 1) * NCH],
                    start=True, stop=True)
                nc.scalar.activation(out=tP[:, kt, :, :], in_=sc_t[:],
                                     func=Exp, scale=0.125)

            def emit_out(qc, h, attnT, ap, half):
                """PV matmul (M=65 incl. the ones/denominator column),
                copy to SBUF, DMA the raw block out."""
                pv = pv_p.tile([DK + 1, NCH], F32, tag="pv",
                               name=f"pv{ap}_{h}")
                for kt in range(KT):
                    nc.tensor.matmul(
                        pv[:],
                        vP[:, kt, h * (DK + 1):(h + 1) * (DK + 1)],
                        attnT[:, kt, half, :],
                        start=(kt == 0), stop=(kt == KT - 1))
                ot_sb = ot_p.tile([DK + 1, NCH], F32, tag="ot",
                                  name=f"ot{ap}_{h}")
                nc.vector.tensor_copy(out=ot_sb[:], in_=pv[:])
                nc.sync.dma_start(
                    out=out[h, :, qc * NCH:(qc + 1) * NCH], in_=ot_sb[:])

            def pv_pair(ap):
                hp, qc = iters[ap]
                oh = hp // 4
                drain_through([("v", oh, sc, s4)
                               for sc in (0, 1) for s4 in range(4)])
                tP = attns.pop(ap)
                emit_out(qc, 2 * hp, tP, ap, 0)
                emit_out(qc, 2 * hp + 1, tP, ap, 1)

            pvi = [0]

            def maybe_pv():
                if pvi[0] < len(iters):
                    pv_pair(pvi[0])
                    pvi[0] += 1

            for ap, (hp, qc) in enumerate(iters):
                for kt in range(KT):
                    if kt == 0:
                        drain_through([("k", hp, 0), ("q", hp, qc)])
                    elif kt == 4:
                        drain_through([("k", hp, 1)])
                    sc_step(ap, hp, qc, kt)
                    if kt in (1, 3, 5):
                        drain_n(1)
                    if kt == 2 and ap >= PV_LAG:
                        maybe_pv()   # mid-iter PV spreads the PE load
                if ap >= len(iters) - 2:   # shrink the end-of-kernel PV tail
                    maybe_pv()
            drain_n(len(queue))
            while pvi[0] < len(iters):
                maybe_pv()
            assert qpos[0] == len(queue) and not attns

    nc.finalize()
    return nc


def _get_program():
    key = "prog"
    if key not in _cache:
        _cache[key] = _build()
    return _cache[key]


def _prep_in_maps(inputs):
    import ml_dtypes

    BF = ml_dtypes.bfloat16

    def prep_w(W):
        # W [D_OUT, D_IN] -> W.T [D_IN, D_OUT] -> [oh, t, p, s] DMA-native
        wT = np.asarray(W, dtype=np.float32).T.astype(BF)
        return np.ascontiguousarray(
            wT.reshape(KT, P, 2, NCH).transpose(2, 0, 1, 3))

    def prep_x(x):
        # x [S, D_IN] -> x.T [D_IN, S] -> [t, p, s] DMA-native
        return np.ascontiguousarray(
            np.asarray(x, dtype=np.float32).T.astype(BF).reshape(KT, P, S))

    query = np.asarray(inputs["query"], dtype=np.float32)
    key_ = np.asarray(inputs["key_"], dtype=np.float32)
    value = np.asarray(inputs["value"], dtype=np.float32)
    wqT, wkT, wvT = (prep_w(inputs[k]) for k in ("Wq", "Wk", "Wv"))
    bq = np.ascontiguousarray(np.asarray(inputs["bq"], dtype=np.float32))
    bk = np.ascontiguousarray(np.asarray(inputs["bk"], dtype=np.float32))
    bv = np.ascontiguousarray(np.asarray(inputs["bv"], dtype=np.float32))
    return [
        {
            "xqT": prep_x(query[b]),
            "xkT": prep_x(key_[b]),
            "xvT": prep_x(value[b]),
            "wqT": wqT, "wkT": wkT, "wvT": wvT,
            "bq": bq, "bk": bk, "bv": bv,
        }
        for b in range(B)
    ]


def kernel(query, key_, value, Wq, bq, Wk, bk, Wv, bv):
    from concourse.bass_utils import run_bass_kernel_spmd

    nc = _get_program()
    in_maps = _prep_in_maps(dict(
        query=query, key_=key_, value=value,
        Wq=Wq, bq=bq, Wk=Wk, bk=bk, Wv=Wv, bv=bv,
    ))
    res = run_bass_kernel_spmd(nc, in_maps, list(range(B)))
    out = np.empty((B, S * D), dtype=np.float32)
    for b in range(B):
        o = np.asarray(res.results[b]["out"])          # [H, DK+1, S]
        x = o[:, :DK, :] / o[:, DK:DK + 1, :]          # [H, DK, S]
        out[b] = x.transpose(2, 0, 1).reshape(-1)      # [S, H*DK] flattened
    return out


# revision 45
# speedup vs baseline: 1.3936x; 1.0617x over previous
"""Multi-head attention kernel for Trainium2, 8 NeuronCores, data-parallel over batch.

Problem (matches the reference nn.Module):
  B=8, S=1024, D_IN=D_OUT=1024, H=16, D_K=64, fp32 in/out.
  q/k/v = Linear(x) per input; scores = q k^T / sqrt(64); attn = softmax;
  out = (attn v) heads-concatenated -> [B, S*D_OUT].

Strategy:
  - One batch element per core (8 cores). No collectives.
  - All matmul operands bf16 (PSUM f32). Host pre-transposes inputs.
  - On-chip layouts:
      kT/qT [P, KT, S]  (o = p + 128*t; head h lives at t=h//2,
                         partitions (h%2)*64..+64)
      vP    [P, KT, H*(DK+1)]  per-head 65-col groups, col 64 = 1.0 so
                         the PV matmul also emits the softmax denominator.
  - Scores are emitted as HEAD PAIRS: even head h uses array rows 0-63,
    odd head h+1 rows 64-127 (tile_position auto-derived from base
    partitions) -> the two 64-contraction matmuls run CONCURRENTLY in
    the PE array (row tiling), ~2x scores throughput.
  - attn^T = exp(scores^T/8) on ACT; ACT is the near-bottleneck
    (~147us of exp), so the schedule starts exp as early as possible:
    warmup MMs trip the HAM clock gate + preload the exp table while
    the first K/Q o-tiles stream in; scores for head pair 0 start after
    only 3 projection blocks; all remaining projection blocks are
    interleaved between scores steps to keep the PE busy while ACT
    drains the exp backlog.
  - pv[65, q] = V'_h.T @ attn^T accumulated over 8 k-tiles; raw blocks
    (incl. denominator row 64) are DMA'd out; the final transpose to
    [q, h*64+d] and the divide happen on the HOST.
"""

import numpy as np

B = 8
S = 1024
D = 1024          # D_IN == D_OUT
H = 16
DK = 64           # D_K
KT = 8            # 128-row tiles along a 1024 dim
QC = 2            # q-chunks of 512
P = 128
NCH = 512         # matmul moving free dim
PV_LAG = 3        # pair-iterations between scores emission and PV
N_WARM = 10       # HAM warmup matmuls (~4.3us @ cold clock)

_cache = {}


def _build():
    import concourse.tile as tile
    import concourse.mybir as mybir
    from concourse import bacc

    F32 = mybir.dt.float32
    BF16 = mybir.dt.bfloat16
    Exp = mybir.ActivationFunctionType.Exp
    MMDT = BF16

    nc = bacc.Bacc(None, target_bir_lowering=False, debug=True)

    # host pre-tiles inputs so every DMA chunk is fully contiguous in DRAM
    # (128-256 KB per descriptor -> full per-queue HBM streaming rate)
    xqT = nc.declare_dram_parameter("xqT", [KT, P, S], BF16, isOutput=False)
    xkT = nc.declare_dram_parameter("xkT", [KT, P, S], BF16, isOutput=False)
    xvT = nc.declare_dram_parameter("xvT", [KT, P, S], BF16, isOutput=False)
    wqT = nc.declare_dram_parameter("wqT", [2, KT, P, NCH], BF16, isOutput=False)
    wkT = nc.declare_dram_parameter("wkT", [2, KT, P, NCH], BF16, isOutput=False)
    wvT = nc.declare_dram_parameter("wvT", [2, KT, P, NCH], BF16, isOutput=False)
    bq = nc.declare_dram_parameter("bq", [D], F32, isOutput=False)
    bk = nc.declare_dram_parameter("bk", [D], F32, isOutput=False)
    bv = nc.declare_dram_parameter("bv", [D], F32, isOutput=False)
    # raw per-head PV output + separate softmax denominators;
    # host divides + transposes
    out = nc.declare_dram_parameter("out", [H, DK, S], F32, isOutput=True)
    den = nc.declare_dram_parameter("den", [H, S], F32, isOutput=True)

    with tile.TileContext(nc) as tc:
        with tc.tile_pool(name="persist", bufs=1) as persist, \
             tc.tile_pool(name="attn_p", bufs=PV_LAG + 1) as attn_p, \
             tc.tile_pool(name="ot_p", bufs=2) as ot_p, \
             tc.tile_pool(name="scr_p", bufs=2) as scr_p, \
             tc.tile_pool(name="x_p", bufs=3) as x_p, \
             tc.tile_pool(name="w_p", bufs=3) as w_p, \
             tc.tile_pool(name="b_p", bufs=1) as b_p, \
             tc.tile_pool(name="wu_p", bufs=1) as wu_p, \
             tc.tile_pool(name="pp", bufs=2, space="PSUM") as pp, \
             tc.tile_pool(name="sc_p", bufs=2, space="PSUM") as sc_p, \
             tc.tile_pool(name="pv_p", bufs=1, space="PSUM") as pv_p, \
             tc.tile_pool(name="dn_ps", bufs=1, space="PSUM") as dn_ps:

            qT = persist.tile([P, KT, S], MMDT, tag="qT")
            kT = persist.tile([P, KT, S], MMDT, tag="kT")
            vP = persist.tile([P, KT, H * DK], MMDT, tag="vP")

            # ---- input tiles ----
            # Pre-roll loads round-robin sync/gpsimd queues (a single queue
            # sustains only ~170 GB/s, which starved the kernel start);
            # mid-kernel loads stay on sync so the gpsimd queue is free for
            # the bias adds.  NEVER dispatch loads on the scalar queue: a
            # blocked dma_start there stalls the whole exp stream.
            xsrc = {"q": xqT, "k": xkT, "v": xvT}
            wsrc = {"q": wqT, "k": wkT, "v": wvT}
            xtiles, wtiles = {}, {}

            def get_x(kind, eng=None):
                # full-width [P, KT, S] tile, 256 KB contiguous per t-chunk
                if kind not in xtiles:
                    t_ = x_p.tile([P, KT, S], MMDT, tag="x", name=f"x_{kind}")
                    for t in range(KT):
                        (eng or nc.sync).dma_start(
                            out=t_[:, t, :], in_=xsrc[kind][t, :, :])
                    xtiles[kind] = t_
                return xtiles[kind]

            def get_w(kind, oh, eng=None):
                key = (kind, oh)
                if key not in wtiles:
                    t_ = w_p.tile([P, KT, NCH], MMDT, tag="w",
                                  name=f"w_{kind}{oh}")
                    for t in range(KT):
                        (eng or nc.sync).dma_start(
                            out=t_[:, t, :], in_=wsrc[kind][oh, t, :, :])
                    wtiles[key] = t_
                return wtiles[key]

            # prefetch the pre-roll inputs before anything else queues
            get_x("k", eng=nc.sync), get_w("k", 0, eng=nc.gpsimd)
            get_x("q", eng=nc.sync), get_w("q", 0, eng=nc.gpsimd)

            # ---- biases ----
            bqs = b_p.tile([P, KT], F32, tag="bqs")
            bks = b_p.tile([P, KT], F32, tag="bks")
            nc.sync.dma_start(out=bqs[:], in_=bq[:].rearrange("(t p) -> p t", p=P))
            nc.sync.dma_start(out=bks[:], in_=bk[:].rearrange("(t p) -> p t", p=P))
            bvb = b_p.tile([P, D], F32, tag="bvb")
            nc.gpsimd.dma_start(out=bvb[:], in_=bv[:].partition_broadcast(P))

            # all-ones stationary column for the M=1 denominator matmuls
            onesb = b_p.tile([P, 1], MMDT, tag="onesb")
            nc.vector.memset(onesb[:], 1.0)

            # ---- HAM warmup + exp table preload ----
            wu = wu_p.tile([P, NCH], MMDT, tag="wu")
            nc.vector.memset(wu[:], 0.0)
            wuf = wu_p.tile([P, 8], F32, tag="wuf")
            nc.vector.memset(wuf[:], 0.0)
            wue = wu_p.tile([P, 8], F32, tag="wue")
            nc.scalar.activation(out=wue[:], in_=wuf[:], func=Exp, scale=1.0)
            for i in range(N_WARM):
                ps_ = pp.tile([P, NCH], F32, tag="proj", name=f"warm{i}")
                nc.tensor.matmul(ps_[:], wu[:, 0:P], wu[:],
                                 start=True, stop=True)

            # ---- projection blocks ----
            # biases run on the (otherwise idle) gpsimd engine so the proj
            # psum ring never waits behind DVE tree/copy backlog
            def kq_block(kind, ot, sc):
                dst, bias = (qT, bqs) if kind == "q" else (kT, bks)
                w_sb = get_w(kind, ot // 4)
                x_sb = get_x(kind)
                o4 = ot % 4
                ps_ = pp.tile([P, NCH], F32, tag="proj",
                              name=f"ps_{kind}{ot}{sc}")
                for it in range(KT):
                    nc.tensor.matmul(
                        ps_[:],
                        w_sb[:, it, o4 * P:(o4 + 1) * P],
                        x_sb[:, it, sc * NCH:(sc + 1) * NCH],
                        start=(it == 0), stop=(it == KT - 1))
                nc.vector.tensor_scalar_add(
                    out=dst[:, ot, sc * NCH:(sc + 1) * NCH],
                    in0=ps_[:], scalar1=bias[:, ot:ot + 1])

            def v_block(oh, sc, s4):
                w_sb = get_w("v", oh)
                x_sb = get_x("v")
                st = sc * 4 + s4
                ps_ = pp.tile([P, NCH], F32, tag="proj", name=f"ps_v{oh}{st}")
                for it in range(KT):
                    nc.tensor.matmul(
                        ps_[:],
                        x_sb[:, it, sc * NCH + s4 * P:sc * NCH + (s4 + 1) * P],
                        w_sb[:, it, :],
                        start=(it == 0), stop=(it == KT - 1))
                nc.vector.tensor_tensor(
                    out=vP[:, st, oh * NCH:(oh + 1) * NCH],
                    in0=ps_[:],
                    in1=bvb[:, oh * NCH:(oh + 1) * NCH],
                    op=mybir.AluOpType.add)

            def run_block(bid):
                if bid[0] == "v":
                    v_block(bid[1], bid[2], bid[3])
                else:
                    kq_block(bid[0], bid[1], bid[2])
                done_ids.add(bid)

            # pre-roll blocks emitted before the first scores step
            # (scores kb 0-1 only need the first K s-half; K01 is force-
            # drained at kb==2, shortening the startup critical path)
            pre = [("k", 0, 0), ("q", 0, 0)]
            queue = [("k", 0, 1), ("q", 0, 1)]
            for ot in (1,):
                queue += [("k", ot, 0), ("k", ot, 1), ("q", ot, 0), ("q", ot, 1)]
            queue += [("v", 0, sc, s4) for sc in (0, 1) for s4 in range(4)]
            for ot in (2, 3, 4):
                queue += [("k", ot, 0), ("k", ot, 1), ("q", ot, 0), ("q", ot, 1)]
            queue += [("v", 1, sc, s4) for sc in (0, 1) for s4 in range(4)]
            for ot in (5, 6, 7):
                queue += [("k", ot, 0), ("k", ot, 1), ("q", ot, 0), ("q", ot, 1)]

            done_ids = set()
            qpos = [0]

            def drain_n(n):
                took = 0
                while qpos[0] < len(queue) and took < n:
                    run_block(queue[qpos[0]])
                    qpos[0] += 1
                    took += 1

            def drain_through(ids):
                while any(i not in done_ids for i in ids):
                    assert qpos[0] < len(queue), f"missing {ids}"
                    run_block(queue[qpos[0]])
                    qpos[0] += 1

            for b in pre:
                run_block(b)

            # ---- attention emission ----
            iters = [(hp, qc) for hp in range(H // 2) for qc in range(QC)]
            attns = {}

            def sc_step(ap, hp, qc, kt):
                """One k-tile of scores for BOTH heads of the pair.  Both
                matmuls write ONE psum tile and one exp call consumes it, so
                their WAR readiness is simultaneous -> the Tile scheduler
                issues them adjacently -> the PE runs them concurrently in
                disjoint row groups (even head rows 0-63, odd head 64-127)."""
                if kt == 0:
                    attns[ap] = attn_p.tile([P, KT, 2, NCH], MMDT,
                                            tag="attnT", name=f"a{ap}")
                tP = attns[ap]
                sc_t = sc_p.tile([P, 2, NCH], F32, tag="sc",
                                 name=f"sc{ap}_{kt}")
                nc.tensor.matmul(
                    sc_t[:, 0, :],
                    kT[0:DK, hp, kt * P:(kt + 1) * P],
                    qT[0:DK, hp, qc * NCH:(qc + 1) * NCH],
                    start=True, stop=True)
                nc.tensor.matmul(
                    sc_t[:, 1, :],
                    kT[DK:P, hp, kt * P:(kt + 1) * P],
                    qT[DK:P, hp, qc * NCH:(qc + 1) * NCH],
                    start=True, stop=True)
                nc.scalar.activation(out=tP[:, kt, :, :], in_=sc_t[:],
                                     func=Exp, scale=0.125)

            pending_dve = []   # small tree chunks, drained 1 per sc_step
            den_q = []         # pairs whose denominator matmuls are due

            def tree_chunks(tP, half, s_):
                """kt-reduction attnT[:, :, half, :] -> s_[:, 0, :], as four
                small DVE ops (<=0.7us each) so interleaved bias adds are
                never stuck behind a long DVE op."""
                A = mybir.AluOpType.add

                def c1a():
                    nc.vector.tensor_tensor(out=s_[:, 0:2, :],
                                            in0=tP[:, 0:2, half, :],
                                            in1=tP[:, 4:6, half, :], op=A)

                def c1b():
                    nc.vector.tensor_tensor(out=s_[:, 2:4, :],
                                            in0=tP[:, 2:4, half, :],
                                            in1=tP[:, 6:8, half, :], op=A)

                def c2():
                    nc.vector.tensor_tensor(out=s_[:, 0:2, :],
                                            in0=s_[:, 0:2, :],
                                            in1=s_[:, 2:4, :], op=A)

                def c3():
                    nc.vector.tensor_tensor(out=s_[:, 0:1, :],
                                            in0=s_[:, 0:1, :],
                                            in1=s_[:, 1:2, :], op=A)
                pending_dve.extend([c1a, c1b, c2, c3])

            def pv_pair(ap):
                hp, qc = iters[ap]
                oh = hp // 4
                drain_through([("v", oh, sc, s4)
                               for sc in (0, 1) for s4 in range(4)])
                tP = attns.pop(ap)
                hA, hB = 2 * hp, 2 * hp + 1
                # col-tiled pair: head A -> array cols/psum parts 0-63,
                # head B -> 64-127.  Both MMs of each kt read the same attnT
                # tile and write the same psum tile, so their readiness is
                # simultaneous -> adjacent issue -> concurrent execution.
                pv = pv_p.tile([P, NCH], F32, tag="pv", name=f"pv{ap}")
                for kt in range(KT):
                    nc.tensor.matmul(
                        pv[0:DK, :],
                        vP[:, kt, hA * DK:(hA + 1) * DK], tP[:, kt, 0, :],
                        start=(kt == 0), stop=(kt == KT - 1),
                        skip_group_check=True)
                    nc.tensor.matmul(
                        pv[DK:P, :],
                        vP[:, kt, hB * DK:(hB + 1) * DK], tP[:, kt, 1, :],
                        start=(kt == 0), stop=(kt == KT - 1),
                        skip_group_check=True)
                ot_sb = ot_p.tile([P, NCH], F32, tag="ot", name=f"ot{ap}")
                nc.vector.tensor_copy(out=ot_sb[:], in_=pv[:])
                nc.sync.dma_start(
                    out=out[hA, :, qc * NCH:(qc + 1) * NCH], in_=ot_sb[0:DK, :])
                nc.sync.dma_start(
                    out=out[hB, :, qc * NCH:(qc + 1) * NCH], in_=ot_sb[DK:P, :])
                sA = scr_p.tile([P, 4, NCH], MMDT, tag="scr", name=f"sA{ap}")
                sB = scr_p.tile([P, 4, NCH], MMDT, tag="scr", name=f"sB{ap}")
                tree_chunks(tP, 0, sA)
                tree_chunks(tP, 1, sB)
                den_q.append((ap, sA, sB))

            def emit_den():
                ap, sA, sB = den_q.pop(0)
                hp, qc = iters[ap]
                dn = dn_ps.tile([33, NCH], F32, tag="dn", name=f"dn{ap}")
                nc.tensor.matmul(dn[0:1, :], onesb[:], sA[:, 0, :],
                                 start=True, stop=True)
                nc.tensor.matmul(dn[32:33, :], onesb[:], sB[:, 0, :],
                                 start=True, stop=True)
                dsb = ot_p.tile([33, NCH], F32, tag="dsb", name=f"dsb{ap}")
                nc.vector.tensor_copy(out=dsb[:], in_=dn[:])
                nc.sync.dma_start(
                    out=den[2 * hp, qc * NCH:(qc + 1) * NCH], in_=dsb[0:1, :])
                nc.sync.dma_start(
                    out=den[2 * hp + 1, qc * NCH:(qc + 1) * NCH],
                    in_=dsb[32:33, :])

            pvi = [0]

            def maybe_pv():
                if pvi[0] < len(iters):
                    pv_pair(pvi[0])
                    pvi[0] += 1

            for ap, (hp, qc) in enumerate(iters):
                for kt in range(KT):
                    if kt == 0:
                        drain_through([("k", hp, 0), ("q", hp, qc)])
                        if ap >= PV_LAG:
                            maybe_pv()
                    elif kt == 4:
                        drain_through([("k", hp, 1)])
                        if ap >= len(iters) - 2:   # shrink the PV tail
                            maybe_pv()
                    sc_step(ap, hp, qc, kt)
                    if kt in (1, 3, 5):
                        drain_n(1)
                    if pending_dve:
                        pending_dve.pop(0)()
                # denominators for the pair(s) PV'd this iteration; all tree
                # chunks must be EMITTED before the den matmuls that read them
                if den_q:
                    while pending_dve:
                        pending_dve.pop(0)()
                    while den_q:
                        emit_den()
            drain_n(len(queue))
            while pvi[0] < len(iters):
                maybe_pv()
            while pending_dve:
                pending_dve.pop(0)()
            while den_q:
                emit_den()
            assert qpos[0] == len(queue) and not attns

    nc.finalize()
    return nc


def _get_program():
    key = "prog"
    if key not in _cache:
        _cache[key] = _build()
    return _cache[key]


def _prep_in_maps(inputs):
    import ml_dtypes

    BF = ml_dtypes.bfloat16

    def prep_w(W):
        # W [D_OUT, D_IN] -> W.T [D_IN, D_OUT] -> [oh, t, p, s] DMA-native
        wT = np.asarray(W, dtype=np.float32).T.astype(BF)
        return np.ascontiguousarray(
            wT.reshape(KT, P, 2, NCH).transpose(2, 0, 1, 3))

    def prep_x(x):
        # x [S, D_IN] -> x.T [D_IN, S] -> [t, p, s] DMA-native
        return np.ascontiguousarray(
            np.asarray(x, dtype=np.float32).T.astype(BF).reshape(KT, P, S))

    query = np.asarray(inputs["query"], dtype=np.float32)
    key_ = np.asarray(inputs["key_"], dtype=np.float32)
    value = np.asarray(inputs["value"], dtype=np.float32)
    wqT, wkT, wvT = (prep_w(inputs[k]) for k in ("Wq", "Wk", "Wv"))
    bq = np.ascontiguousarray(np.asarray(inputs["bq"], dtype=np.float32))
    bk = np.ascontiguousarray(np.asarray(inputs["bk"], dtype=np.float32))
    bv = np.ascontiguousarray(np.asarray(inputs["bv"], dtype=np.float32))
    return [
        {
            "xqT": prep_x(query[b]),
            "xkT": prep_x(key_[b]),
            "xvT": prep_x(value[b]),
            "wqT": wqT, "wkT": wkT, "wvT": wvT,
            "bq": bq, "bk": bk, "bv": bv,
        }
        for b in range(B)
    ]


def kernel(query, key_, value, Wq, bq, Wk, bk, Wv, bv):
    from concourse.bass_utils import run_bass_kernel_spmd

    nc = _get_program()
    in_maps = _prep_in_maps(dict(
        query=query, key_=key_, value=value,
        Wq=Wq, bq=bq, Wk=Wk, bk=bk, Wv=Wv, bv=bv,
    ))
    res = run_bass_kernel_spmd(nc, in_maps, list(range(B)))
    out = np.empty((B, S * D), dtype=np.float32)
    for b in range(B):
        o = np.asarray(res.results[b]["out"])          # [H, DK, S]
        dn = np.asarray(res.results[b]["den"])         # [H, S]
        x = o / dn[:, None, :]                         # [H, DK, S]
        out[b] = x.transpose(2, 0, 1).reshape(-1)      # [S, H*DK] flattened
    return out
